# revision 30
# baseline (speedup 1.0000x reference)
"""Trainium2 Bass kernel for nn_BsplineLoss (chamfer between skeletal points
and bspline curve points).

Full-input contract: kernel(**inputs) takes the unsharded arrays
  skeletal_points      (16, 4096, 3) f32
  primitive_parameters (16, 64, 12)  f32
  bspline_basis        (16, 4)       f32
and returns the full (16,) f32 loss.

Sharding: data-parallel over batch B=16 across 8 cores (2 batches/core).

Device algorithm (per core, per batch):
  curves b = einsum(basis, ctrl)           (M=1024 points)
  psum'[p,m] = 2*a_p.b_m - |b_m|^2         (one K=6 matmul per p-chunk; the
                                            three "ones" lhsT rows pick up the
                                            -|b|^2 rows of the rhs)
  rowmax[p]  = max_m psum'                 -> rowmin_d2 = |a_p|^2 - rowmax
  ncmin[r,m] = max_chunks (psum' - |a|^2)  -> colmin_d2 = -max_partitions ncmin
Host: relu, sqrt, mean, add -> loss.
"""

import numpy as np

P = 128
NB = 2          # batches per core
NCHUNK = 32     # p-chunks per batch (chunk j = points {32r + j})
JPP = 32        # points per partition per batch
M = 1024        # curve points per batch
NCORES = 8
USE_TTR = True

_CACHE = {}


def _register_min_op():
    """Register a custom DVE op: out = min(in0, in1); accum_out = min(c0,
    min_k out). Reads two SBUF streams at 1 elem/cycle/lane each — twice the
    fresh-data rate of tensor_reduce for the row-min."""
    from concourse import dve_ops
    from concourse.dve_spec import Spec, minn, Src0, Src1, C0, lower, _has_src1
    from concourse.dve_uop import DveOpSpec

    name = "TT_MIN_RED_ANT"
    for o in dve_ops.OPS:
        if o.name == name:
            return o

    def _ref(in0, in1, c0, c1, c2):
        body = np.minimum(in0.astype(np.float32), in1.astype(np.float32))
        acc = np.minimum(
            c0, body.reshape(body.shape[0], -1).min(axis=-1, keepdims=True)
        )
        return body, acc

    spec = Spec(body=minn(Src0, Src1), accum=minn, accum_init=C0, reference=_ref)
    opcode = max(dve_ops._SUB_OPCODE_FOR_NAME.values()) + 1
    assert opcode < 0x20
    shas = {}
    for ver in ("v3", "v4"):
        try:
            s = DveOpSpec(
                name=name, opcode=opcode, uops=lower(spec, ver=ver),
                rd1_en=_has_src1(spec),
            )
            shas[ver] = s.sha(ver)
        except Exception:
            pass
    op = dve_ops.DveOp(name, spec, subdim=False, uops_sha=shas,
                       perf_en={"v3": True, "v4": True})
    dve_ops.OPS.append(op)
    dve_ops.CUSTOM_DVE_SPECS[name] = spec
    dve_ops._SUB_OPCODE_FOR_NAME[name] = opcode
    return op


def _build_nc():
    import concourse.bacc as bacc
    import concourse.bass as bass
    import concourse.tile as tile
    from concourse import mybir, bass_isa

    f32 = mybir.dt.float32
    bf16 = mybir.dt.bfloat16
    AX = mybir.AxisListType
    AL = mybir.AluOpType
    ACT = mybir.ActivationFunctionType

    min_op = _register_min_op()
    nc = bacc.Bacc(None, target_bir_lowering=False)

    skel = nc.dram_tensor("skel", [NB * 4096, 3], f32, kind="ExternalInput")
    prim = nc.dram_tensor("prim", [P, 12], f32, kind="ExternalInput")
    basis = nc.dram_tensor("basis", [16, 4], f32, kind="ExternalInput")

    orow = nc.dram_tensor("orow", [P, NB * NCHUNK], f32, kind="ExternalOutput")
    ocol = nc.dram_tensor("ocol", [NB, M], f32, kind="ExternalOutput")

    scratch = nc.dram_tensor("scratch", [P, 128], bf16)
    scratch_a = nc.dram_tensor("scratch_a", [NB, P, 13 * JPP], bf16)

    ident_dram = nc.inline_tensor(np.eye(P, dtype=np.float32), name="ident")

    with tile.TileContext(nc) as tc:
        with (
            tc.tile_pool(name="const", bufs=1) as constp,
            tc.tile_pool(name="prep", bufs=2) as prep,
            tc.tile_pool(name="persist", bufs=1) as persist,
        ):
            lh6 = persist.tile([13, NB, P, NCHUNK], bf16)
            a2pos = persist.tile([P, NB * NCHUNK], f32)

            def emit_aside(b):
                # asr rows: 0-2 a_hi, 3-5 a_lo, 6-8 a_hi, 9-10 ones, 11-12 a2_hi/lo;
                # DRAM bounce so the reload puts g on partitions (j-contiguous
                # 64B runs); per-chunk lhsT slices are strided (stride NCHUNK)
                as2 = prep.tile([P, JPP, 3], f32, tag="as2")
                nc.sync.dma_start(
                    as2[:],
                    skel.rearrange("(b r j) c -> b r (j c)", b=NB, r=P, j=JPP)[b],
                )
                sqa = prep.tile([P, JPP, 3], f32, tag="sqa")
                nc.scalar.square(sqa[:], as2[:])
                nc.vector.tensor_reduce(
                    a2pos[:, b * NCHUNK : (b + 1) * NCHUNK],
                    sqa[:],
                    axis=AX.X,
                    op=AL.add,
                )
                asr = prep.tile([P, 13, JPP], bf16, tag="asr")
                nc.vector.memset(asr[:], 1.0)
                ah_v = asr[:, 0:3, :].rearrange("r c j -> r j c")
                nc.vector.tensor_copy(ah_v, as2[:])
                nc.vector.tensor_copy(
                    asr[:, 6:9, :].rearrange("r c j -> r j c"), as2[:]
                )
                nc.vector.tensor_tensor(
                    out=asr[:, 3:6, :].rearrange("r c j -> r j c"),
                    in0=as2[:],
                    in1=ah_v,
                    op=AL.subtract,
                )
                a2s = a2pos[:, b * NCHUNK : (b + 1) * NCHUNK]
                nc.vector.tensor_copy(asr[:, 11, :], a2s)
                nc.vector.tensor_tensor(
                    out=asr[:, 12, :], in0=a2s, in1=asr[:, 11, :], op=AL.subtract
                )
                nc.gpsimd.dma_start(scratch_a[b], asr[:])
                dmae = nc.scalar if b == 0 else nc.sync
                dmae.dma_start(
                    lh6[:, b],
                    scratch_a[b].rearrange("r (g j) -> g r j", g=13, j=JPP),
                )

            with tc.tile_pool(name="pprep", bufs=2, space="PSUM") as pprep:
                ident = constp.tile([P, P], f32)
                nc.sync.dma_start(ident[:], ident_dram[:])

                emit_aside(0)

                # ---------- B side: curve points -> RHS (11, 2048) ---------
                # B6[3n+c, 16c+t] = 2*basis[t, n]  (block-diagonal over c)
                b6 = persist.tile([12, 48], f32)
                nc.vector.memset(b6[:], 0.0)
                for c in range(3):
                    for n in range(4):
                        nc.sync.dma_start(
                            b6[3 * n + c : 3 * n + c + 1, 16 * c : 16 * c + 16],
                            basis[:, n : n + 1],
                        )
                nc.scalar.mul(b6[:], b6[:], 2.0)

                pp = prep.tile([P, 12], f32)
                nc.sync.dma_start(pp[:], prim[:])
                ps_cpt = pprep.tile([12, P], f32)
                nc.tensor.transpose(ps_cpt[:], pp[:], ident[:])
                cpt = prep.tile([12, P], f32)
                nc.scalar.copy(cpt[:], ps_cpt[:])

                ps_cv = pprep.tile([P, 48], f32)
                nc.tensor.matmul(ps_cv[:], cpt[:], b6[:])  # (128,48) = 2*curves

                # sb bf16 (P,128): [0:48]=R0=bf16(2b), [48:96]=R1=2b-R0,
                # [96:112]=(-b^2)_hi, [112:128]=(-b^2)_lo
                sb = prep.tile([P, 128], bf16)
                nc.scalar.copy(sb[:, 0:48], ps_cv[:])
                nc.vector.tensor_tensor(
                    out=sb[:, 48:96], in0=ps_cv[:], in1=sb[:, 0:48], op=AL.subtract
                )
                sq = prep.tile([P, 48], f32)
                nc.scalar.activation(sq[:], ps_cv[:], ACT.Square, scale=0.5)
                nb2 = prep.tile([P, 16], f32)
                nc.vector.tensor_reduce(
                    nb2[:],
                    sq[:].rearrange("p (c t) -> p t c", c=3, t=16),
                    axis=AX.X,
                    op=AL.add,
                    negate=True,
                )
                nc.vector.tensor_copy(sb[:, 96:112], nb2[:])
                nc.vector.tensor_tensor(
                    out=sb[:, 112:128], in0=nb2[:], in1=sb[:, 96:112], op=AL.subtract
                )

                nc.gpsimd.dma_start(scratch[:], sb[:])
                rhs = persist.tile([13, NB * M], bf16)
                nc.vector.memset(rhs[:], -1.0)   # rows 11-12 stay -1
                r0_src = scratch[:, 0:48].rearrange("q (c t) -> c q t", c=3, t=16)
                r1_src = scratch[:, 48:96].rearrange("q (c t) -> c q t", c=3, t=16)
                nc.sync.dma_start(rhs[0:3, :], r0_src)
                nc.sync.dma_start(rhs[3:6, :], r0_src)
                nc.sync.dma_start(rhs[6:9, :], r1_src)
                nc.sync.dma_start(rhs[9:10, :], scratch[:, 96:112])
                nc.sync.dma_start(rhs[10:11, :], scratch[:, 112:128])

            # ---------------- main loop --------------------------------
            with (
                tc.tile_pool(name="mpsum", bufs=2, space="PSUM") as mpsum,
                tc.tile_pool(name="mout", bufs=1) as mout,
                tc.tile_pool(name="cmin2", bufs=4) as cmin2,
            ):
                # sbd = Relu(-psum' + |a|^2) = max(d2, 0);
                # rowraw[:, col] = min_m sbd = rowmin_d2
                rowraw = mout.tile([P, NB * NCHUNK], f32)
                HM = M // 2

                def emit_main(b):
                    for jj in range(0, NCHUNK, 2):
                        ps_d = mpsum.tile([P, 2 * M], f32, tag="psd")
                        sbd = cmin2.tile([P, 2 * M], bf16, tag="sbd")
                        for u in range(2):
                            lhsT = lh6[:, b, :, jj + u]
                            for h2 in range(2):
                                nc.tensor.matmul(
                                    ps_d[:, u * M + h2 * 512 : u * M + (h2 + 1) * 512],
                                    lhsT,
                                    rhs[:, b * M + h2 * 512 : b * M + (h2 + 1) * 512],
                                )
                        # psum'' = -d2; one constant-bias drain for both chunks
                        nc.scalar.activation(
                            sbd[:], ps_d[:], ACT.Relu, bias=0.0, scale=-1.0
                        )
                        for u in range(2):
                            col = b * NCHUNK + jj + u
                            pair = cmin2.tile([P, HM], bf16, tag="pair")
                            nc.vector._custom_dve(
                                min_op,
                                out=pair[:],
                                in0=sbd[:, u * M : u * M + HM],
                                in1=sbd[:, u * M + HM : (u + 1) * M],
                                s0=3.0e38,
                                accum_out=rowraw[:, col : col + 1],
                            )
                            new = cmin2.tile([P, M], bf16, tag="cmin")
                            nc.vector.tensor_tensor(
                                out=new[:],
                                in0=sbd[:, u * M : (u + 1) * M],
                                in1=prev_box[0][:],
                                op=AL.min,
                            )
                            prev_box[0] = new
                    # negate so the gpsimd fold can use max (no min support)
                    cmf = cmin2.tile([P, M], f32, tag="cmf")
                    nc.vector.tensor_scalar_mul(cmf[:], prev_box[0][:], -1.0)
                    go = cmin2.tile([P, M], f32, tag="gpout")
                    nc.gpsimd.partition_all_reduce(
                        go[:], cmf[:], channels=P, reduce_op=bass_isa.ReduceOp.max
                    )
                    nc.sync.dma_start(ocol[b : b + 1, :], go[0:1, :])
                    prev_box[0] = None

                def emit_batch_init():
                    prev = cmin2.tile([P, M], bf16, tag="cmin")
                    nc.vector.memset(prev[:], 3.0e38)
                    return [prev]

                prev_box = emit_batch_init()
                emit_main(0)
                emit_aside(1)
                prev_box = emit_batch_init()
                emit_main(1)

                nc.sync.dma_start(orow[:], rowraw[:])

    nc.compile()
    return nc


def _get_nc():
    if "nc" not in _CACHE:
        _CACHE["nc"] = _build_nc()
    return _CACHE["nc"]


def make_in_maps(skeletal_points, primitive_parameters, bspline_basis):
    skel = np.ascontiguousarray(skeletal_points, dtype=np.float32)
    prim = np.ascontiguousarray(primitive_parameters, dtype=np.float32)
    basis = np.ascontiguousarray(bspline_basis, dtype=np.float32)
    in_maps = []
    for c in range(NCORES):
        sk = skel[NB * c : NB * (c + 1)].reshape(NB * 4096, 3)
        pr = prim[NB * c : NB * (c + 1)].reshape(P, 12)
        in_maps.append(
            {
                "skel": np.ascontiguousarray(sk),
                "prim": np.ascontiguousarray(pr),
                "basis": basis,
            }
        )
    return in_maps


def postprocess(results):
    """results: list of 8 per-core dicts with orow/oa2/ocol."""
    loss = np.zeros(16, dtype=np.float32)
    for c, r in enumerate(results):
        rowmax = r["orow"].astype(np.float64)   # (128, 64)
        ocol = r["ocol"].astype(np.float64)     # (2, 1024)
        for b in range(NB):
            rm = rowmax[:, b * NCHUNK : (b + 1) * NCHUNK]
            # (128, 32) rowmin_d2 (already relu'd), point p = 32r + j
            cha = np.sqrt(np.maximum(rm, 0.0)).mean()
            cm = -ocol[b]
            chb = np.sqrt(np.maximum(cm, 0.0)).mean()
            loss[NB * c + b] = np.float32(cha + chb)
    return loss


def kernel(skeletal_points, primitive_parameters, bspline_basis):
    from concourse.bass_utils import run_bass_kernel_spmd

    nc = _get_nc()
    in_maps = make_in_maps(skeletal_points, primitive_parameters, bspline_basis)
    res = run_bass_kernel_spmd(nc, in_maps, core_ids=list(range(NCORES)))
    return postprocess(res.results)


# revision 31
# speedup vs baseline: 1.0142x; 1.0142x over previous
"""Trainium2 Bass kernel for nn_BsplineLoss (chamfer between skeletal points
and bspline curve points).

Full-input contract: kernel(**inputs) takes the unsharded arrays
  skeletal_points      (16, 4096, 3) f32
  primitive_parameters (16, 64, 12)  f32
  bspline_basis        (16, 4)       f32
and returns the full (16,) f32 loss.

Sharding: data-parallel over batch B=16 across 8 cores (2 batches/core).

Device algorithm (per core, per batch):
  curves b = einsum(basis, ctrl)           (M=1024 points)
  psum'[p,m] = 2*a_p.b_m - |b_m|^2         (one K=6 matmul per p-chunk; the
                                            three "ones" lhsT rows pick up the
                                            -|b|^2 rows of the rhs)
  rowmax[p]  = max_m psum'                 -> rowmin_d2 = |a_p|^2 - rowmax
  ncmin[r,m] = max_chunks (psum' - |a|^2)  -> colmin_d2 = -max_partitions ncmin
Host: relu, sqrt, mean, add -> loss.
"""

import numpy as np

P = 128
NB = 2          # batches per core
NCHUNK = 32     # p-chunks per batch (chunk j = points {32r + j})
JPP = 32        # points per partition per batch
M = 1024        # curve points per batch
NCORES = 8
USE_TTR = True

_CACHE = {}


def _register_min_op():
    """Register a custom DVE op: out = min(in0, in1); accum_out = min(c0,
    min_k out). Reads two SBUF streams at 1 elem/cycle/lane each — twice the
    fresh-data rate of tensor_reduce for the row-min."""
    from concourse import dve_ops
    from concourse.dve_spec import Spec, minn, Src0, Src1, C0, lower, _has_src1
    from concourse.dve_uop import DveOpSpec

    name = "TT_MIN_RED_ANT"
    for o in dve_ops.OPS:
        if o.name == name:
            return o

    def _ref(in0, in1, c0, c1, c2):
        body = np.minimum(in0.astype(np.float32), in1.astype(np.float32))
        acc = np.minimum(
            c0, body.reshape(body.shape[0], -1).min(axis=-1, keepdims=True)
        )
        return body, acc

    spec = Spec(body=minn(Src0, Src1), accum=minn, accum_init=C0, reference=_ref)
    opcode = max(dve_ops._SUB_OPCODE_FOR_NAME.values()) + 1
    assert opcode < 0x20
    shas = {}
    for ver in ("v3", "v4"):
        try:
            s = DveOpSpec(
                name=name, opcode=opcode, uops=lower(spec, ver=ver),
                rd1_en=_has_src1(spec),
            )
            shas[ver] = s.sha(ver)
        except Exception:
            pass
    op = dve_ops.DveOp(name, spec, subdim=False, uops_sha=shas,
                       perf_en={"v3": True, "v4": True})
    dve_ops.OPS.append(op)
    dve_ops.CUSTOM_DVE_SPECS[name] = spec
    dve_ops._SUB_OPCODE_FOR_NAME[name] = opcode
    return op


def _build_nc():
    import concourse.bacc as bacc
    import concourse.bass as bass
    import concourse.tile as tile
    from concourse import mybir, bass_isa

    f32 = mybir.dt.float32
    bf16 = mybir.dt.bfloat16
    AX = mybir.AxisListType
    AL = mybir.AluOpType
    ACT = mybir.ActivationFunctionType

    min_op = _register_min_op()
    nc = bacc.Bacc(None, target_bir_lowering=False)

    skel = nc.dram_tensor("skel", [NB * 4096, 3], f32, kind="ExternalInput")
    prim = nc.dram_tensor("prim", [P, 12], f32, kind="ExternalInput")
    basis = nc.dram_tensor("basis", [16, 4], f32, kind="ExternalInput")

    orow = nc.dram_tensor("orow", [P, NB * NCHUNK], f32, kind="ExternalOutput")
    ocol = nc.dram_tensor("ocol", [NB, M], f32, kind="ExternalOutput")

    scratch = nc.dram_tensor("scratch", [P, 128], bf16)
    scratch_a = nc.dram_tensor("scratch_a", [NB, P, 13 * JPP], bf16)

    ident_dram = nc.inline_tensor(np.eye(P, dtype=np.float32), name="ident")

    with tile.TileContext(nc) as tc:
        with (
            tc.tile_pool(name="const", bufs=1) as constp,
            tc.tile_pool(name="prep", bufs=2) as prep,
            tc.tile_pool(name="persist", bufs=1) as persist,
        ):
            lh6 = persist.tile([13, NB, P, NCHUNK], bf16)
            a2pos = persist.tile([P, NB * NCHUNK], f32)

            def emit_aside(b):
                # asr rows: 0-2 a_hi, 3-5 a_lo, 6-8 a_hi, 9-10 ones, 11-12 a2_hi/lo;
                # DRAM bounce so the reload puts g on partitions (j-contiguous
                # 64B runs); per-chunk lhsT slices are strided (stride NCHUNK)
                ldq = nc.sync if b == 0 else nc.gpsimd
                as2 = prep.tile([P, JPP, 3], f32, tag="as2")
                ldq.dma_start(
                    as2[:],
                    skel.rearrange("(b r j) c -> b r (j c)", b=NB, r=P, j=JPP)[b],
                )
                sqa = prep.tile([P, JPP, 3], f32, tag="sqa")
                nc.scalar.square(sqa[:], as2[:])
                nc.vector.tensor_reduce(
                    a2pos[:, b * NCHUNK : (b + 1) * NCHUNK],
                    sqa[:],
                    axis=AX.X,
                    op=AL.add,
                )
                asr = prep.tile([P, 13, JPP], bf16, tag="asr")
                nc.vector.memset(asr[:], 1.0)
                ah_v = asr[:, 0:3, :].rearrange("r c j -> r j c")
                nc.vector.tensor_copy(ah_v, as2[:])
                nc.vector.tensor_copy(
                    asr[:, 6:9, :].rearrange("r c j -> r j c"), as2[:]
                )
                nc.vector.tensor_tensor(
                    out=asr[:, 3:6, :].rearrange("r c j -> r j c"),
                    in0=as2[:],
                    in1=ah_v,
                    op=AL.subtract,
                )
                a2s = a2pos[:, b * NCHUNK : (b + 1) * NCHUNK]
                nc.vector.tensor_copy(asr[:, 11, :], a2s)
                nc.vector.tensor_tensor(
                    out=asr[:, 12, :], in0=a2s, in1=asr[:, 11, :], op=AL.subtract
                )
                nc.gpsimd.dma_start(scratch_a[b], asr[:])
                dmae = nc.scalar if b == 0 else nc.gpsimd
                dmae.dma_start(
                    lh6[:, b],
                    scratch_a[b].rearrange("r (g j) -> g r j", g=13, j=JPP),
                )

            with tc.tile_pool(name="pprep", bufs=2, space="PSUM") as pprep:
                ident = constp.tile([P, P], f32)
                nc.scalar.dma_start(ident[:], ident_dram[:])

                emit_aside(0)

                # ---------- B side: curve points -> RHS (11, 2048) ---------
                # B6[3n+c, 16c+t] = 2*basis[t, n]  (block-diagonal over c)
                b6 = persist.tile([12, 48], f32)
                nc.vector.memset(b6[:], 0.0)
                for c in range(3):
                    for n in range(4):
                        nc.scalar.dma_start(
                            b6[3 * n + c : 3 * n + c + 1, 16 * c : 16 * c + 16],
                            basis[:, n : n + 1],
                        )
                nc.scalar.mul(b6[:], b6[:], 2.0)

                pp = prep.tile([P, 12], f32)
                nc.sync.dma_start(pp[:], prim[:])
                ps_cpt = pprep.tile([12, P], f32)
                nc.tensor.transpose(ps_cpt[:], pp[:], ident[:])
                cpt = prep.tile([12, P], f32)
                nc.scalar.copy(cpt[:], ps_cpt[:])

                ps_cv = pprep.tile([P, 48], f32)
                nc.tensor.matmul(ps_cv[:], cpt[:], b6[:])  # (128,48) = 2*curves

                # sb bf16 (P,128): [0:48]=R0=bf16(2b), [48:96]=R1=2b-R0,
                # [96:112]=(-b^2)_hi, [112:128]=(-b^2)_lo
                sb = prep.tile([P, 128], bf16)
                nc.scalar.copy(sb[:, 0:48], ps_cv[:])
                nc.vector.tensor_tensor(
                    out=sb[:, 48:96], in0=ps_cv[:], in1=sb[:, 0:48], op=AL.subtract
                )
                sq = prep.tile([P, 48], f32)
                nc.scalar.activation(sq[:], ps_cv[:], ACT.Square, scale=0.5)
                nb2 = prep.tile([P, 16], f32)
                nc.vector.tensor_reduce(
                    nb2[:],
                    sq[:].rearrange("p (c t) -> p t c", c=3, t=16),
                    axis=AX.X,
                    op=AL.add,
                    negate=True,
                )
                nc.vector.tensor_copy(sb[:, 96:112], nb2[:])
                nc.vector.tensor_tensor(
                    out=sb[:, 112:128], in0=nb2[:], in1=sb[:, 96:112], op=AL.subtract
                )

                nc.gpsimd.dma_start(scratch[:], sb[:])
                rhs = persist.tile([13, NB * M], bf16)
                nc.vector.memset(rhs[:], -1.0)   # rows 11-12 stay -1
                r0_src = scratch[:, 0:48].rearrange("q (c t) -> c q t", c=3, t=16)
                r1_src = scratch[:, 48:96].rearrange("q (c t) -> c q t", c=3, t=16)
                nc.sync.dma_start(rhs[0:3, :], r0_src)
                nc.scalar.dma_start(rhs[3:6, :], r0_src)
                nc.sync.dma_start(rhs[6:9, :], r1_src)
                nc.scalar.dma_start(rhs[9:10, :], scratch[:, 96:112])
                nc.scalar.dma_start(rhs[10:11, :], scratch[:, 112:128])

            # ---------------- main loop --------------------------------
            with (
                tc.tile_pool(name="mpsum", bufs=2, space="PSUM") as mpsum,
                tc.tile_pool(name="mout", bufs=1) as mout,
                tc.tile_pool(name="cmin2", bufs=4) as cmin2,
            ):
                # sbd = Relu(-psum' + |a|^2) = max(d2, 0);
                # rowraw[:, col] = min_m sbd = rowmin_d2
                rowraw = mout.tile([P, NB * NCHUNK], f32)
                HM = M // 2

                def emit_main(b):
                    for jj in range(0, NCHUNK, 2):
                        ps_d = mpsum.tile([P, 2 * M], f32, tag="psd")
                        sbd = cmin2.tile([P, 2 * M], bf16, tag="sbd")
                        for u in range(2):
                            lhsT = lh6[:, b, :, jj + u]
                            for h2 in range(2):
                                nc.tensor.matmul(
                                    ps_d[:, u * M + h2 * 512 : u * M + (h2 + 1) * 512],
                                    lhsT,
                                    rhs[:, b * M + h2 * 512 : b * M + (h2 + 1) * 512],
                                )
                        # psum'' = -d2; one constant-bias drain for both chunks
                        nc.scalar.activation(
                            sbd[:], ps_d[:], ACT.Relu, bias=0.0, scale=-1.0
                        )
                        for u in range(2):
                            col = b * NCHUNK + jj + u
                            pair = cmin2.tile([P, HM], bf16, tag="pair")
                            nc.vector._custom_dve(
                                min_op,
                                out=pair[:],
                                in0=sbd[:, u * M : u * M + HM],
                                in1=sbd[:, u * M + HM : (u + 1) * M],
                                s0=3.0e38,
                                accum_out=rowraw[:, col : col + 1],
                            )
                            new = cmin2.tile([P, M], bf16, tag="cmin")
                            nc.vector.tensor_tensor(
                                out=new[:],
                                in0=sbd[:, u * M : (u + 1) * M],
                                in1=prev_box[0][:],
                                op=AL.min,
                            )
                            prev_box[0] = new
                    # negate so the gpsimd fold can use max (no min support)
                    cmf = cmin2.tile([P, M], f32, tag="cmf")
                    nc.vector.tensor_scalar_mul(cmf[:], prev_box[0][:], -1.0)
                    go = cmin2.tile([P, M], f32, tag="gpout")
                    nc.gpsimd.partition_all_reduce(
                        go[:], cmf[:], channels=P, reduce_op=bass_isa.ReduceOp.max
                    )
                    nc.sync.dma_start(ocol[b : b + 1, :], go[0:1, :])
                    prev_box[0] = None

                def emit_batch_init():
                    prev = cmin2.tile([P, M], bf16, tag="cmin")
                    nc.vector.memset(prev[:], 3.0e38)
                    return [prev]

                prev_box = emit_batch_init()
                emit_main(0)
                emit_aside(1)
                prev_box = emit_batch_init()
                emit_main(1)

                nc.sync.dma_start(orow[:], rowraw[:])

    nc.compile()
    return nc


def _get_nc():
    if "nc" not in _CACHE:
        _CACHE["nc"] = _build_nc()
    return _CACHE["nc"]


def make_in_maps(skeletal_points, primitive_parameters, bspline_basis):
    skel = np.ascontiguousarray(skeletal_points, dtype=np.float32)
    prim = np.ascontiguousarray(primitive_parameters, dtype=np.float32)
    basis = np.ascontiguousarray(bspline_basis, dtype=np.float32)
    in_maps = []
    for c in range(NCORES):
        sk = skel[NB * c : NB * (c + 1)].reshape(NB * 4096, 3)
        pr = prim[NB * c : NB * (c + 1)].reshape(P, 12)
        in_maps.append(
            {
                "skel": np.ascontiguousarray(sk),
                "prim": np.ascontiguousarray(pr),
                "basis": basis,
            }
        )
    return in_maps


def postprocess(results):
    """results: list of 8 per-core dicts with orow/oa2/ocol."""
    loss = np.zeros(16, dtype=np.float32)
    for c, r in enumerate(results):
        rowmax = r["orow"].astype(np.float64)   # (128, 64)
        ocol = r["ocol"].astype(np.float64)     # (2, 1024)
        for b in range(NB):
            rm = rowmax[:, b * NCHUNK : (b + 1) * NCHUNK]
            # (128, 32) rowmin_d2 (already relu'd), point p = 32r + j
            cha = np.sqrt(np.maximum(rm, 0.0)).mean()
            cm = -ocol[b]
            chb = np.sqrt(np.maximum(cm, 0.0)).mean()
            loss[NB * c + b] = np.float32(cha + chb)
    return loss


def kernel(skeletal_points, primitive_parameters, bspline_basis):
    from concourse.bass_utils import run_bass_kernel_spmd

    nc = _get_nc()
    in_maps = make_in_maps(skeletal_points, primitive_parameters, bspline_basis)
    res = run_bass_kernel_spmd(nc, in_maps, core_ids=list(range(NCORES)))
    return postprocess(res.results)


# revision 33
# speedup vs baseline: 1.0455x; 1.0308x over previous
"""Trainium2 Bass kernel for nn_BsplineLoss (chamfer between skeletal points
and bspline curve points).

Full-input contract: kernel(**inputs) takes the unsharded arrays
  skeletal_points      (16, 4096, 3) f32
  primitive_parameters (16, 64, 12)  f32
  bspline_basis        (16, 4)       f32
and returns the full (16,) f32 loss.

Sharding: data-parallel over batch B=16 across 8 cores (2 batches/core).

Device algorithm (per core, per batch):
  curves b = einsum(basis, ctrl)           (M=1024 points)
  psum'[p,m] = 2*a_p.b_m - |b_m|^2         (one K=6 matmul per p-chunk; the
                                            three "ones" lhsT rows pick up the
                                            -|b|^2 rows of the rhs)
  rowmax[p]  = max_m psum'                 -> rowmin_d2 = |a_p|^2 - rowmax
  ncmin[r,m] = max_chunks (psum' - |a|^2)  -> colmin_d2 = -max_partitions ncmin
Host: relu, sqrt, mean, add -> loss.
"""

import numpy as np

P = 128
NB = 2          # batches per core
NCHUNK = 32     # p-chunks per batch (chunk j = points {32r + j})
JPP = 32        # points per partition per batch
M = 1024        # curve points per batch
NCORES = 8
USE_TTR = True

_CACHE = {}


def _register_min_op():
    """Register a custom DVE op: out = min(in0, in1); accum_out = min(c0,
    min_k out). Reads two SBUF streams at 1 elem/cycle/lane each — twice the
    fresh-data rate of tensor_reduce for the row-min."""
    from concourse import dve_ops
    from concourse.dve_spec import Spec, minn, Src0, Src1, C0, lower, _has_src1
    from concourse.dve_uop import DveOpSpec

    name = "TT_MIN_RED_ANT"
    for o in dve_ops.OPS:
        if o.name == name:
            return o

    def _ref(in0, in1, c0, c1, c2):
        body = np.minimum(in0.astype(np.float32), in1.astype(np.float32))
        acc = np.minimum(
            c0, body.reshape(body.shape[0], -1).min(axis=-1, keepdims=True)
        )
        return body, acc

    spec = Spec(body=minn(Src0, Src1), accum=minn, accum_init=C0, reference=_ref)
    opcode = max(dve_ops._SUB_OPCODE_FOR_NAME.values()) + 1
    assert opcode < 0x20
    shas = {}
    for ver in ("v3", "v4"):
        try:
            s = DveOpSpec(
                name=name, opcode=opcode, uops=lower(spec, ver=ver),
                rd1_en=_has_src1(spec),
            )
            shas[ver] = s.sha(ver)
        except Exception:
            pass
    op = dve_ops.DveOp(name, spec, subdim=False, uops_sha=shas,
                       perf_en={"v3": True, "v4": True})
    dve_ops.OPS.append(op)
    dve_ops.CUSTOM_DVE_SPECS[name] = spec
    dve_ops._SUB_OPCODE_FOR_NAME[name] = opcode
    return op


def _build_nc():
    import concourse.bacc as bacc
    import concourse.bass as bass
    import concourse.tile as tile
    from concourse import mybir, bass_isa

    f32 = mybir.dt.float32
    bf16 = mybir.dt.bfloat16
    AX = mybir.AxisListType
    AL = mybir.AluOpType
    ACT = mybir.ActivationFunctionType

    min_op = _register_min_op()
    nc = bacc.Bacc(None, target_bir_lowering=False)

    skel = nc.dram_tensor("skel", [NB * 4096, 3], f32, kind="ExternalInput")
    prim = nc.dram_tensor("prim", [P, 12], f32, kind="ExternalInput")
    basis = nc.dram_tensor("basis", [16, 4], f32, kind="ExternalInput")

    orow = nc.dram_tensor("orow", [P, NB * NCHUNK], f32, kind="ExternalOutput")
    ocol = nc.dram_tensor("ocol", [NB, M], f32, kind="ExternalOutput")

    scratch = nc.dram_tensor("scratch", [P, 128], bf16)
    scratch_a = nc.dram_tensor("scratch_a", [NB, P, 13 * JPP], bf16)

    ident_dram = nc.inline_tensor(np.eye(P, dtype=np.float32), name="ident")

    with tile.TileContext(nc) as tc:
        with (
            tc.tile_pool(name="const", bufs=1) as constp,
            tc.tile_pool(name="prep", bufs=2) as prep,
            tc.tile_pool(name="persist", bufs=1) as persist,
        ):
            lh6 = persist.tile([13, NB, P, NCHUNK], bf16)
            a2pos = persist.tile([P, NB * NCHUNK], f32)

            def emit_aside(b):
                # asr rows: 0-2 a_hi, 3-5 a_lo, 6-8 a_hi, 9-10 ones, 11-12 a2_hi/lo;
                # DRAM bounce so the reload puts g on partitions (j-contiguous
                # 64B runs); per-chunk lhsT slices are strided (stride NCHUNK)
                ldq = nc.sync if b == 0 else nc.gpsimd
                as2 = prep.tile([P, JPP, 3], f32, tag="as2")
                ldq.dma_start(
                    as2[:],
                    skel.rearrange("(b r j) c -> b r (j c)", b=NB, r=P, j=JPP)[b],
                )
                sqa = prep.tile([P, JPP, 3], f32, tag="sqa")
                nc.scalar.square(sqa[:], as2[:])
                nc.vector.tensor_reduce(
                    a2pos[:, b * NCHUNK : (b + 1) * NCHUNK],
                    sqa[:],
                    axis=AX.X,
                    op=AL.add,
                )
                asr = prep.tile([P, 13, JPP], bf16, tag="asr")
                nc.vector.memset(asr[:], 1.0)
                ah_v = asr[:, 0:3, :].rearrange("r c j -> r j c")
                nc.vector.tensor_copy(ah_v, as2[:])
                nc.vector.tensor_copy(
                    asr[:, 6:9, :].rearrange("r c j -> r j c"), as2[:]
                )
                nc.vector.tensor_tensor(
                    out=asr[:, 3:6, :].rearrange("r c j -> r j c"),
                    in0=as2[:],
                    in1=ah_v,
                    op=AL.subtract,
                )
                a2s = a2pos[:, b * NCHUNK : (b + 1) * NCHUNK]
                nc.vector.tensor_copy(asr[:, 11, :], a2s)
                nc.vector.tensor_tensor(
                    out=asr[:, 12, :], in0=a2s, in1=asr[:, 11, :], op=AL.subtract
                )
                nc.gpsimd.dma_start(scratch_a[b], asr[:])
                dmae = nc.scalar if b == 0 else nc.gpsimd
                dmae.dma_start(
                    lh6[:, b],
                    scratch_a[b].rearrange("r (g j) -> g r j", g=13, j=JPP),
                )

            with tc.tile_pool(name="pprep", bufs=2, space="PSUM") as pprep:
                ident = constp.tile([P, P], f32)
                nc.scalar.dma_start(ident[:], ident_dram[:])

                emit_aside(0)

                # ---------- B side: curve points -> RHS (11, 2048) ---------
                # B6[3n+c, 16c+t] = 2*basis[t, n]  (block-diagonal over c)
                b6 = persist.tile([12, 48], f32)
                nc.vector.memset(b6[:], 0.0)
                _qs = [nc.sync, nc.scalar, nc.gpsimd]
                for c in range(3):
                    for n in range(4):
                        _qs[(3 * n + c) % 3].dma_start(
                            b6[3 * n + c : 3 * n + c + 1, 16 * c : 16 * c + 16],
                            basis[:, n : n + 1],
                        )
                nc.scalar.mul(b6[:], b6[:], 2.0)

                pp = prep.tile([P, 12], f32)
                nc.sync.dma_start(pp[:], prim[:])
                ps_cpt = pprep.tile([12, P], f32)
                nc.tensor.transpose(ps_cpt[:], pp[:], ident[:])
                cpt = prep.tile([12, P], f32)
                nc.scalar.copy(cpt[:], ps_cpt[:])

                ps_cv = pprep.tile([P, 48], f32)
                nc.tensor.matmul(ps_cv[:], cpt[:], b6[:])  # (128,48) = 2*curves

                # sb bf16 (P,128): [0:48]=R0=bf16(2b), [48:96]=R1=2b-R0,
                # [96:112]=(-b^2)_hi, [112:128]=(-b^2)_lo
                sb = prep.tile([P, 128], bf16)
                nc.scalar.copy(sb[:, 0:48], ps_cv[:])
                nc.vector.tensor_tensor(
                    out=sb[:, 48:96], in0=ps_cv[:], in1=sb[:, 0:48], op=AL.subtract
                )
                sq = prep.tile([P, 48], f32)
                nc.scalar.activation(sq[:], ps_cv[:], ACT.Square, scale=0.5)
                nb2 = prep.tile([P, 16], f32)
                nc.vector.tensor_reduce(
                    nb2[:],
                    sq[:].rearrange("p (c t) -> p t c", c=3, t=16),
                    axis=AX.X,
                    op=AL.add,
                    negate=True,
                )
                nc.vector.tensor_copy(sb[:, 96:112], nb2[:])
                nc.vector.tensor_tensor(
                    out=sb[:, 112:128], in0=nb2[:], in1=sb[:, 96:112], op=AL.subtract
                )

                nc.gpsimd.dma_start(scratch[:], sb[:])
                rhs = persist.tile([13, NB * M], bf16)
                nc.vector.memset(rhs[:], -1.0)   # rows 11-12 stay -1
                r0_src = scratch[:, 0:48].rearrange("q (c t) -> c q t", c=3, t=16)
                r1_src = scratch[:, 48:96].rearrange("q (c t) -> c q t", c=3, t=16)
                nc.sync.dma_start(rhs[0:3, :], r0_src)
                nc.scalar.dma_start(rhs[3:6, :], r0_src)
                nc.sync.dma_start(rhs[6:9, :], r1_src)
                nc.scalar.dma_start(rhs[9:10, :], scratch[:, 96:112])
                nc.scalar.dma_start(rhs[10:11, :], scratch[:, 112:128])

            # ---------------- main loop --------------------------------
            with (
                tc.tile_pool(name="mpsum", bufs=2, space="PSUM") as mpsum,
                tc.tile_pool(name="mout", bufs=1) as mout,
                tc.tile_pool(name="cmin2", bufs=4) as cmin2,
            ):
                # sbd = Relu(-psum' + |a|^2) = max(d2, 0);
                # rowraw[:, col] = min_m sbd = rowmin_d2
                rowraw = mout.tile([P, NB * NCHUNK], f32)
                HM = M // 2

                def emit_main(b):
                    for jj in range(0, NCHUNK, 2):
                        ps_d = mpsum.tile([P, 2 * M], f32, tag="psd")
                        sbd = cmin2.tile([P, 2 * M], bf16, tag="sbd")
                        for u in range(2):
                            lhsT = lh6[:, b, :, jj + u]
                            for h2 in range(2):
                                nc.tensor.matmul(
                                    ps_d[:, u * M + h2 * 512 : u * M + (h2 + 1) * 512],
                                    lhsT,
                                    rhs[:, b * M + h2 * 512 : b * M + (h2 + 1) * 512],
                                )
                        # psum'' = -d2; one constant-bias drain for both chunks
                        nc.scalar.activation(
                            sbd[:], ps_d[:], ACT.Relu, bias=0.0, scale=-1.0
                        )
                        for u in range(2):
                            col = b * NCHUNK + jj + u
                            pair = cmin2.tile([P, HM], bf16, tag="pair")
                            nc.vector._custom_dve(
                                min_op,
                                out=pair[:],
                                in0=sbd[:, u * M : u * M + HM],
                                in1=sbd[:, u * M + HM : (u + 1) * M],
                                s0=3.0e38,
                                accum_out=rowraw[:, col : col + 1],
                            )
                            new = cmin2.tile([P, M], bf16, tag="cmin")
                            nc.vector.tensor_tensor(
                                out=new[:],
                                in0=sbd[:, u * M : (u + 1) * M],
                                in1=prev_box[0][:],
                                op=AL.min,
                            )
                            prev_box[0] = new
                    # negate so the gpsimd fold can use max (no min support)
                    cmf = cmin2.tile([P, M], f32, tag="cmf")
                    nc.vector.tensor_scalar_mul(cmf[:], prev_box[0][:], -1.0)
                    go = cmin2.tile([P, M], f32, tag="gpout")
                    nc.gpsimd.partition_all_reduce(
                        go[:], cmf[:], channels=P, reduce_op=bass_isa.ReduceOp.max
                    )
                    nc.sync.dma_start(ocol[b : b + 1, :], go[0:1, :])
                    prev_box[0] = None

                def emit_batch_init():
                    prev = cmin2.tile([P, M], bf16, tag="cmin")
                    nc.vector.memset(prev[:], 3.0e38)
                    return [prev]

                prev_box = emit_batch_init()
                emit_main(0)
                emit_aside(1)
                prev_box = emit_batch_init()
                emit_main(1)

                nc.sync.dma_start(orow[:], rowraw[:])

    nc.compile()
    return nc


def _get_nc():
    if "nc" not in _CACHE:
        _CACHE["nc"] = _build_nc()
    return _CACHE["nc"]


def make_in_maps(skeletal_points, primitive_parameters, bspline_basis):
    skel = np.ascontiguousarray(skeletal_points, dtype=np.float32)
    prim = np.ascontiguousarray(primitive_parameters, dtype=np.float32)
    basis = np.ascontiguousarray(bspline_basis, dtype=np.float32)
    in_maps = []
    for c in range(NCORES):
        sk = skel[NB * c : NB * (c + 1)].reshape(NB * 4096, 3)
        pr = prim[NB * c : NB * (c + 1)].reshape(P, 12)
        in_maps.append(
            {
                "skel": np.ascontiguousarray(sk),
                "prim": np.ascontiguousarray(pr),
                "basis": basis,
            }
        )
    return in_maps


def postprocess(results):
    """results: list of 8 per-core dicts with orow/oa2/ocol."""
    loss = np.zeros(16, dtype=np.float32)
    for c, r in enumerate(results):
        rowmax = r["orow"].astype(np.float64)   # (128, 64)
        ocol = r["ocol"].astype(np.float64)     # (2, 1024)
        for b in range(NB):
            rm = rowmax[:, b * NCHUNK : (b + 1) * NCHUNK]
            # (128, 32) rowmin_d2 (already relu'd), point p = 32r + j
            cha = np.sqrt(np.maximum(rm, 0.0)).mean()
            cm = -ocol[b]
            chb = np.sqrt(np.maximum(cm, 0.0)).mean()
            loss[NB * c + b] = np.float32(cha + chb)
    return loss


def kernel(skeletal_points, primitive_parameters, bspline_basis):
    from concourse.bass_utils import run_bass_kernel_spmd

    nc = _get_nc()
    in_maps = make_in_maps(skeletal_points, primitive_parameters, bspline_basis)
    res = run_bass_kernel_spmd(nc, in_maps, core_ids=list(range(NCORES)))
    return postprocess(res.results)
